# revision 36
# baseline (speedup 1.0000x reference)
"""Causal dot-product attention for Trainium2 (Bass/Tile), 8-core SPMD.

Problem: B=32, T=2048, D=64 fp32.  reference:
    O = softmax(mask(Q K^T / sqrt(D))) V      (causal mask, per batch)

Sharding: pure batch parallelism - 4 batches per NeuronCore, no collectives.

v4: fp16 operands end-to-end on the PE (1 cyc/col), exp column count at the
exact causal minimum, host-side epilogue.  CPU-sim rel err ~4e-4.

Per-core algorithm (flash-style; no online rescale: scores ~ N(0,1), exp is
computed directly with a constant stability shift that cancels in softmax):

  S^T layout (= K Q^T) so the PV contraction (over key positions) lands on
  the partition dim and the softmax sums ride along for free as a
  ones-column of V (row 64 of the transposed PV accumulator).

  The S^T contraction dim is only D=64, so key chunks are packed into the
  two 64-row halves of the PE array (tile_position row packing, derived
  from operand base partitions) and stream concurrently - concurrent
  matmuls MUST target different PSUM banks.  Host-side prep supplies Q^T
  duplicated into both partition halves and K^T with even/odd chunks
  interleaved (fp16), plus the ones-augmented V (fp16, pre-chunked to the
  device layout so DMA is contiguous).

  Per batch (16 key chunks of 128, 4 query tiles of 512):
    off-diagonal chunk pair u: S^T pair -> [128,1024] of a PSUM tile
      (2 banks), one 1024-col exp pass PSUM->SBUF (fp16), 2 PV matmuls.
    the 4 diagonal chunks are packed into ONE [128,1280] score tile with
      per-chunk query offsets so only causally-live columns are computed:
        cols [0,512)      chunk 4i   (q_local 0:512)    bank 0
        cols [512,896)    chunk 4i+1 (q_local 128:512)  bank 1
        cols [896,1024)   chunk 4i+3 (q_local 384:512)  bank 1
        cols [1024,1280)  chunk 4i+2 (q_local 256:512)  bank 2
      -> ONE 1280-col exp pass; every mask is then a plain 128-wide
      lower-triangle multiply (DVE); 4 PV matmuls.
    Total exp columns/batch = 17408 = the causal minimum.
    epilogue per q-tile: DVE copy of the raw O^T accumulator [65,512]
      (row 64 = softmax denominators) PSUM->SBUF, DMA out.  Normalization
      and the transpose back to [q, 64] happen on the host.
"""

import os

# Standard recovery knob: reset NeuronCores at runtime init (harmless on a
# healthy device, helps if a previous run left cores wedged).
os.environ.setdefault("NEURON_RT_RESET_CORES", "1")

import numpy as np

import concourse.bacc as bacc
import concourse.mybir as mybir
import concourse.tile as tile
from concourse.bass_utils import run_bass_kernel_spmd

B, T, D = 32, 2048, 64
NCORES = 8
BL = B // NCORES            # batches per core
P = 128                     # partitions / key-chunk size
NCH = T // P                # key chunks per batch (16)
QW = 512                    # query-tile width
NQT = T // QW               # query tiles per batch (4)
SCALE = 1.0 / np.sqrt(D)    # 0.125
EBIAS = -2.0                # stability shift inside exp(); cancels in softmax

F32 = mybir.dt.float32
F16 = mybir.dt.float16
BF16 = mybir.dt.bfloat16


def build_nc():
    from contextlib import ExitStack

    nc = bacc.Bacc()
    # host-prepped inputs (fp16):
    #   q2: Q^T duplicated into both partition halves      [BL, 128, T]
    #   k2: K^T, even chunks rows 0:64, odd rows 64:128    [BL, 128, T/2]
    #   v:  V with ones column, device chunk layout        [BL, 128, NCH, 65]
    q2_d = nc.dram_tensor("q2", [BL, P, T], F16, kind="ExternalInput")
    k2_d = nc.dram_tensor("k2", [BL, P, T // 2], F16, kind="ExternalInput")
    v_d = nc.dram_tensor("v", [BL, P, NCH, D + 1], F16, kind="ExternalInput")
    # raw O^T accumulator tiles; row 64 = softmax denominators.  fp16 is
    # plenty: the host divides rows 0:64 by row 64, so the shared exponent
    # cancels and only the ~5e-4 mantissa rounding survives.
    o_d = nc.dram_tensor("o", [BL, NQT, D + 1, QW], F16, kind="ExternalOutput")

    with tile.TileContext(nc) as tc, ExitStack() as ctx:
        singles = ctx.enter_context(tc.tile_pool(name="singles", bufs=1))
        wpool = ctx.enter_context(tc.tile_pool(name="wts", bufs=4))
        pepool = ctx.enter_context(tc.tile_pool(name="pexp", bufs=12))
        osb_pool = ctx.enter_context(tc.tile_pool(name="osb", bufs=3))
        st_ps = ctx.enter_context(tc.tile_pool(name="stps", bufs=2, space="PSUM"))
        sd_ps = ctx.enter_context(tc.tile_pool(name="sdps", bufs=1, space="PSUM"))
        ot_ps = ctx.enter_context(tc.tile_pool(name="otps", bufs=1, space="PSUM"))

        ebias = singles.tile([P, 1], F32)
        nc.vector.memset(ebias, EBIAS)
        # 0/1 lower-triangle mask (keep where col >= partition), built fp32
        # then copied to fp16; applied by DVE multiplies
        tri0_f = singles.tile([P, P], F32)
        nc.vector.memset(tri0_f, 1.0)
        nc.gpsimd.affine_select(
            out=tri0_f, in_=tri0_f, compare_op=mybir.AluOpType.is_ge, fill=0.0,
            base=0, channel_multiplier=-1, pattern=[[1, P]],
        )
        tri0 = singles.tile([P, P], F16)
        nc.vector.tensor_copy(out=tri0, in_=tri0_f)

        # p-state prewarm: the PE clock ramps to 2.4 GHz only after ~3us of
        # continuous execution.  Run dummy bf16 matmuls during the initial
        # input-DMA wait so the first real (diag-heavy) tiles hit a warm
        # clock; sized to finish right as the first input lands.
        wsrc = singles.tile([P, QW], BF16)
        nc.vector.memset(wsrc, 0.5)
        warm = st_ps.tile([P, 2 * QW], F32, tag="st", name="warm")
        for _ in range(9):
            nc.tensor.matmul(
                out=warm[:, 0:QW], lhsT=wsrc[:, 0:P], rhs=wsrc,
                start=True, stop=True,
            )

        def load_batch(b):
            # first-tile-first slicing for batch 0 (the only exposed load):
            # the i=0 q-tile needs kt cols 0:256, qt cols 0:512, v chunks
            # 0:4.  Later batches prefetch whole during compute.  Spread
            # descriptor generation across idle engine queues.
            qt = wpool.tile([P, T], F16, tag="qt", name=f"qt{b}")
            kt = wpool.tile([P, T // 2], F16, tag="kt", name=f"kt{b}")
            vv = wpool.tile([P, NCH, D + 1], F16, tag="vv", name=f"vv{b}")
            if b == 0:
                # transfer order = descriptor order.  Keep per-partition DMA
                # packets >= 2KB (finer slices were measured DMA-overhead
                # bound).  kt and the first qt half gate the first tiles -
                # issue them on separate engines so descriptor generation
                # overlaps; later qt quarters land just before tiles 2/3
                # need them.
                nc.sync.dma_start(out=kt, in_=k2_d[b])
                nc.gpsimd.dma_start(out=qt[:, 0 : 2 * QW], in_=q2_d[b, :, 0 : 2 * QW])
                nc.sync.dma_start(out=qt[:, 2 * QW : 3 * QW], in_=q2_d[b, :, 2 * QW : 3 * QW])
                nc.gpsimd.dma_start(out=vv, in_=v_d[b])
                nc.sync.dma_start(out=qt[:, 3 * QW :], in_=q2_d[b, :, 3 * QW :])
            else:
                nc.sync.dma_start(out=kt, in_=k2_d[b])
                nc.sync.dma_start(out=qt, in_=q2_d[b])
                nc.gpsimd.dma_start(out=vv, in_=v_d[b])
            return qt, kt, vv


        def s_pair(stp, kt, qt, u, cols, qlo, n):
            """Row-packed S^T chunk pair u: even chunk (PE rows 0:64) and
            odd chunk (rows 64:128) streaming concurrently into different
            PSUM banks.  cols/qlo/n give per-half score-tile column starts,
            query offsets and widths."""
            for h in range(2):
                if n[h] == 0:
                    continue
                nc.tensor.matmul(
                    out=stp[:, cols[h] : cols[h] + n[h]],
                    lhsT=kt[h * D : (h + 1) * D, u * P : (u + 1) * P],
                    rhs=qt[h * D : (h + 1) * D, qlo[h] : qlo[h] + n[h]],
                    start=True,
                    stop=True,
                )

        # ---- flattened pair stream, software-pipelined ----
        # Every (batch, q-tile) contributes its chunk pairs to one global
        # stream.  S-matmuls + exp are emitted W pairs ahead of the trailing
        # PV so a PV stalled on its pexp (or on the accumulator-copy WAR at
        # a tile boundary) never blocks the next score tile in the PE queue,
        # keeping the Activation engine - the bottleneck - saturated.
        W = 3

        class Pair:
            __slots__ = ("b", "i", "kind", "u", "first", "last", "stp", "pexp")
            def __init__(self, b, i, kind, u, first, last):
                self.b, self.i, self.kind, self.u = b, i, kind, u
                self.first, self.last = first, last

        stream = []
        for b in range(BL):
            for i in range(NQT):
                kinds = (
                    [("diag", None)]
                    if i == 0
                    else [("off", 0), ("diag", None)]
                    + [("off", u) for u in range(1, 2 * i)]
                )
                for j, (kind, u) in enumerate(kinds):
                    stream.append(
                        Pair(b, i, kind, u, j == 0, j == len(kinds) - 1)
                    )

        batch_tiles = {}   # b -> (qt, kt, vv)
        tile_otp = {}

        def emit_s_act(p):
            if p.b not in batch_tiles:
                batch_tiles[p.b] = load_batch(p.b)
            qt, kt, vv = batch_tiles[p.b]
            q0 = p.i * QW
            tag = f"{p.b}_{p.i}_{p.u if p.kind == 'off' else 'd'}"
            p.pexp = pepool.tile([P, 3 * QW], F16, tag="pe", name=f"pe{tag}")
            if p.kind == "off":
                p.stp = st_ps.tile([P, 2 * QW], F32, tag="st", name=f"st{tag}")
                s_pair(p.stp, kt, qt, p.u, cols=(0, QW), qlo=(q0, q0), n=(QW, QW))
                nc.scalar.activation(
                    out=p.pexp[:, 0 : 2 * QW],
                    in_=p.stp[:, 0 : 2 * QW],
                    func=mybir.ActivationFunctionType.Exp,
                    bias=ebias,
                    scale=SCALE,
                )
            else:
                # the 4 diagonal chunks in one score tile / one exp pass:
                #   [0,512)     chunk 4i   q_local 0:512    bank 0
                #   [512,896)   chunk 4i+1 q_local 128:512  bank 1
                #   [896,1024)  chunk 4i+3 q_local 384:512  bank 1
                #   [1024,1280) chunk 4i+2 q_local 256:512  bank 2
                p.stp = sd_ps.tile([P, 3 * QW], F32, tag="sd", name=f"st{tag}")
                s_pair(
                    p.stp, kt, qt, 2 * p.i,
                    cols=(0, QW), qlo=(q0, q0 + P), n=(QW, QW - P),
                )
                s_pair(
                    p.stp, kt, qt, 2 * p.i + 1,
                    cols=(2 * QW, QW + 3 * P), qlo=(q0 + 2 * P, q0 + 3 * P),
                    n=(2 * P, P),
                )
                nc.scalar.activation(
                    out=p.pexp[:, 0 : 2 * QW + 2 * P],
                    in_=p.stp[:, 0 : 2 * QW + 2 * P],
                    func=mybir.ActivationFunctionType.Exp,
                    bias=ebias,
                    scale=SCALE,
                )

        def emit_pv(p):
            qt, kt, vv = batch_tiles[p.b]
            key = (p.b, p.i)
            if p.first:
                tile_otp[key] = ot_ps.tile(
                    [P, QW], F32, tag="ot", name=f"ot{p.b}_{p.i}"
                )
            otp = tile_otp[key]

            def pv(chunk, plo, n, olo, start=False, stop=False):
                nc.tensor.matmul(
                    out=otp[0 : D + 1, olo:QW] if olo else otp[0 : D + 1, :],
                    lhsT=vv[:, chunk, :],
                    rhs=p.pexp[:, plo : plo + n],
                    start=start,
                    stop=stop,
                )

            if p.kind == "off":
                pv(2 * p.u, 0, QW, 0, start=p.first)
                pv(2 * p.u + 1, QW, QW, 0, stop=p.last)
            else:
                # per-chunk causal triangles (keep where q-offset >=
                # partition), emitted trailing so they never head-of-line
                # block the previous tile's accumulator cast in the DVE FIFO
                for c0 in (0, QW, QW + 3 * P, 2 * QW):
                    nc.vector.tensor_mul(
                        out=p.pexp[:, c0 : c0 + P],
                        in0=p.pexp[:, c0 : c0 + P],
                        in1=tri0,
                    )
                pv(4 * p.i, 0, QW, 0, start=p.first)
                pv(4 * p.i + 1, QW, QW - P, P)
                pv(4 * p.i + 3, QW + 3 * P, P, 3 * P)
                pv(4 * p.i + 2, 2 * QW, 2 * P, 2 * P, stop=p.last)
            if p.last:
                # raw O^T (+ sums row) to DRAM via a DVE bounce (DMA cannot
                # read PSUM); normalize on host
                osb = osb_pool.tile(
                    [D + 1, QW], F16, tag="osb", name=f"osb{p.b}_{p.i}"
                )
                nc.vector.tensor_copy(out=osb, in_=otp[0 : D + 1, :])
                nc.sync.dma_start(out=o_d[p.b, p.i], in_=osb)

        for j in range(len(stream) + W):
            if j < len(stream):
                emit_s_act(stream[j])
            if j >= W:
                emit_pv(stream[j - W])

    return nc


_NC_CACHE = None


def _get_nc():
    global _NC_CACHE
    if _NC_CACHE is None:
        nc = build_nc()
        nc.finalize()
        _NC_CACHE = nc
    return _NC_CACHE


def prep_inputs(queries, keys, values):
    """Host-side shard + layout prep (numpy only)."""
    q = np.asarray(queries, dtype=np.float32)
    k = np.asarray(keys, dtype=np.float32)
    v = np.asarray(values, dtype=np.float32)
    assert q.shape == (B, T, D), q.shape
    qT = q.transpose(0, 2, 1)                                  # [B, 64, T]
    q2 = np.concatenate([qT, qT], axis=1).astype(np.float16)   # [B, 128, T]
    kT = k.transpose(0, 2, 1).reshape(B, D, NCH, P)            # [B, 64, 16, 128]
    k2 = np.concatenate(
        [
            kT[:, :, 0::2, :].reshape(B, D, T // 2),
            kT[:, :, 1::2, :].reshape(B, D, T // 2),
        ],
        axis=1,
    ).astype(np.float16)                                       # [B, 128, T/2]
    va = np.concatenate(
        [v, np.ones((B, T, 1), np.float32)], axis=-1
    ).astype(np.float16)                                       # [B, T, 65]
    # device layout: partition = key-within-chunk -> [B, 128, NCH, 65]
    va = va.reshape(B, NCH, P, D + 1).transpose(0, 2, 1, 3)
    q2 = np.ascontiguousarray(q2)
    k2 = np.ascontiguousarray(k2)
    va = np.ascontiguousarray(va)
    return [
        {
            "q2": q2[c * BL : (c + 1) * BL],
            "k2": k2[c * BL : (c + 1) * BL],
            "v": va[c * BL : (c + 1) * BL],
        }
        for c in range(NCORES)
    ]


def postprocess(raw):
    """[NCORES][BL, NQT, 65, QW] raw O^T tiles -> [B, T, D] output."""
    o = np.concatenate(raw, axis=0).astype(np.float32)  # [B, NQT, 65, QW]
    out = o[:, :, :D, :] / o[:, :, D : D + 1, :]        # normalize
    out = out.transpose(0, 1, 3, 2).reshape(B, T, D)    # [B, T, D]
    return np.ascontiguousarray(out.astype(np.float32))


def run(queries, keys, values, trace=False):
    nc = _get_nc()
    core_ids = list(range(NCORES))
    in_maps = prep_inputs(queries, keys, values)
    try:
        res = run_bass_kernel_spmd(nc, in_maps, core_ids, trace=trace)
    except Exception:
        # transient NRT_EXEC_UNIT_UNRECOVERABLE has been observed; a
        # straight retry recovers
        res = run_bass_kernel_spmd(nc, in_maps, core_ids, trace=trace)
    out = postprocess([res.results[c]["o"] for c in core_ids])
    return out, res


def kernel(queries, keys, values):
    out, _ = run(queries, keys, values, trace=False)
    return out
